# revision 1
# baseline (speedup 1.0000x reference)
"""MoE kernel for TRN2, 8 NeuronCores, data-parallel over the batch dim.

Reference computation (B=8192, D=1024, H=1024, E=16):
    weights = softmax(x @ Wg + bg, axis=1)            # [B, E]
    h       = relu(einsum('bd,edh->beh', x, W1) + b1) # [B, E, H]
    eo      = einsum('beh,eh->be', h, W2) + b2        # [B, E]
    out     = sum(eo * weights, axis=1, keepdims=True)# [B, 1]

Strategy:
  - Shard B over 8 cores (1024 rows/core); weights replicated.
  - All matmuls contract over the partition dim, so x is transposed on the
    HOST (free) and each core gets xT [D, 1024] resident in SBUF.
  - Stage 1 (per (e, h_tile)=t of 128): psum[h=128, b=512x2] accumulated
    over 8 d-tiles; float32r matmuls (full PE rate at N>=256).
  - ReLU+b1 via ScalarE activation (bias is per-partition in h-major layout).
  - Stage 2: W2 built block-diagonal on host -> every t contributes one
    [128hx16e] @ [128h, 512b] matmul accumulating into ONE [16, 1024] psum
    tile; all 16 experts' outputs land stacked on partitions 0..15.
  - Gating: fp32 matmuls into [128b, 16e] psum; softmax along the free dim;
    exp(bg) folded in multiplicatively (softmax is shift/scale invariant).
  - Combine: eoT + b2 -> PE-transpose 16x128 chunks -> [128b, 16e]; multiply
    by gate weights, reduce along free dim -> [128, 1] -> DMA out.
"""

import numpy as np

import concourse.bacc as bacc
import concourse.bass as bass
import concourse.mybir as mybir
from concourse import tile
from concourse.bass_utils import run_bass_kernel_spmd

B, D, H, E = 8192, 1024, 1024, 16
N_CORES = 8
BS = B // N_CORES  # 1024 batch rows per core
NB = BS // 128     # 8 b-tiles of 128
BH = 512           # half-batch moving-operand width (one psum bank)
DT = D // 128      # 8 d-tiles
HT = H // 128      # 8 h-tiles
T = E * HT         # 128 (e, h_tile) pairs

F32 = mybir.dt.float32
F32R = mybir.dt.float32r
AF = mybir.ActivationFunctionType
AX = mybir.AxisListType


def build_bass():
    nc = bacc.Bacc("TRN2", target_bir_lowering=False, debug=False)
    xt_d = nc.dram_tensor("xt", [D, BS], F32R, kind="ExternalInput")
    w1_d = nc.dram_tensor("w1p", [T, 128, DT * 128], F32R, kind="ExternalInput")
    b1t_d = nc.dram_tensor("b1t", [128, T], F32, kind="ExternalInput")
    w2bd_d = nc.dram_tensor("w2bd", [128, T * E], F32R, kind="ExternalInput")
    wgp_d = nc.dram_tensor("wgp", [128, DT * E], F32R, kind="ExternalInput")
    ebg_d = nc.dram_tensor("ebg", [128, E], F32, kind="ExternalInput")
    b2_d = nc.dram_tensor("b2p", [E, 1], F32, kind="ExternalInput")
    id16_d = nc.dram_tensor("id16", [E, E], F32, kind="ExternalInput")
    y_d = nc.dram_tensor("y", [BS, 1], F32, kind="ExternalOutput")

    with tile.TileContext(nc) as tc:
        with (
            tc.tile_pool(name="const", bufs=1) as cpool,
            tc.tile_pool(name="w1", bufs=4) as w1pool,
            tc.tile_pool(name="hrelu", bufs=4) as hpool,
            tc.tile_pool(name="sm", bufs=2) as smpool,
            tc.tile_pool(name="ps_h", bufs=2, space=bass.MemorySpace.PSUM) as psh,
            tc.tile_pool(name="ps_eo", bufs=1, space=bass.MemorySpace.PSUM) as pseo,
            tc.tile_pool(name="ps_s", bufs=2, space=bass.MemorySpace.PSUM) as pss,
        ):
            # ---- resident tensors ----
            xt_sb = []
            for d in range(DT):
                tl = cpool.tile([128, BS], F32R, tag=f"xt{d}")
                nc.sync.dma_start(tl[:], xt_d[d * 128:(d + 1) * 128, :])
                xt_sb.append(tl)
            w2bd_sb = cpool.tile([128, T * E], F32R, tag="w2bd")
            nc.sync.dma_start(w2bd_sb[:], w2bd_d[:])
            b1t_sb = cpool.tile([128, T], F32, tag="b1t")
            nc.sync.dma_start(b1t_sb[:], b1t_d[:])
            wgp_sb = cpool.tile([128, DT * E], F32R, tag="wgp")
            nc.sync.dma_start(wgp_sb[:], wgp_d[:])
            ebg_sb = cpool.tile([128, E], F32, tag="ebg")
            nc.sync.dma_start(ebg_sb[:], ebg_d[:])
            b2_sb = cpool.tile([E, 1], F32, tag="b2")
            nc.sync.dma_start(b2_sb[:], b2_d[:])
            id16_sb = cpool.tile([E, E], F32, tag="id16")
            nc.sync.dma_start(id16_sb[:], id16_d[:])
            w_all = cpool.tile([128, NB, E], F32, tag="wall")  # gate weights
            eo_sb = cpool.tile([E, BS], F32, tag="eo")         # expert outs ^T

            # ---- gating: logits -> softmax along free dim ----
            for bt in range(NB):
                ps_g = pss.tile([128, E], F32, tag="sps")
                for d in range(DT):
                    nc.tensor.matmul(
                        ps_g[:],
                        xt_sb[d][:, bt * 128:(bt + 1) * 128],
                        wgp_sb[:, d * E:(d + 1) * E],
                        start=(d == 0), stop=(d == DT - 1),
                        skip_group_check=True,
                    )
                pexp = smpool.tile([128, E], F32, tag="pexp")
                nc.scalar.activation(pexp[:], ps_g[:], AF.Exp)
                nc.vector.tensor_mul(pexp[:], pexp[:], ebg_sb[:])
                ssum = smpool.tile([128, 1], F32, tag="ssum")
                nc.vector.reduce_sum(ssum[:], pexp[:], axis=AX.X)
                rsum = smpool.tile([128, 1], F32, tag="rsum")
                nc.vector.reciprocal(rsum[:], ssum[:])
                nc.vector.tensor_scalar_mul(w_all[:, bt, :], pexp[:], rsum[:])

            # ---- main loop over t=(e, h_tile) ----
            eo_ps = pseo.tile([E, BS], F32)

            def emit_stage2(t, hr):
                for bh in range(2):
                    nc.tensor.matmul(
                        eo_ps[:, bh * BH:(bh + 1) * BH],
                        w2bd_sb[:, t * E:(t + 1) * E],
                        hr[:, bh * BH:(bh + 1) * BH],
                        start=(t == 0), stop=(t == T - 1),
                        skip_group_check=True,
                    )

            pending = []
            for t in range(T):
                w1t = w1pool.tile([128, DT * 128], F32R, tag="w1t")
                nc.sync.dma_start(w1t[:], w1_d[t, :, :])
                ps1 = psh.tile([128, BS], F32, tag="ps1")
                for d in range(DT):
                    lhs = w1t[:, d * 128:(d + 1) * 128]
                    for bh in range(2):
                        nc.tensor.matmul(
                            ps1[:, bh * BH:(bh + 1) * BH],
                            lhs,
                            xt_sb[d][:, bh * BH:(bh + 1) * BH],
                            start=(d == 0), stop=(d == DT - 1),
                            skip_group_check=True,
                        )
                if pending:
                    emit_stage2(*pending.pop())
                hr = hpool.tile([128, BS], F32R, tag="hr")
                for bh in range(2):
                    nc.scalar.activation(
                        hr[:, bh * BH:(bh + 1) * BH],
                        ps1[:, bh * BH:(bh + 1) * BH],
                        AF.Relu,
                        bias=b1t_sb[:, t:t + 1],
                    )
                pending.append((t, hr))
            emit_stage2(*pending.pop())

            # ---- combine: (eoT + b2) -> transpose -> * gates -> reduce ----
            nc.vector.tensor_scalar_add(eo_sb[:], eo_ps[:], b2_sb[:])
            for bt in range(NB):
                tps = pss.tile([128, E], F32, tag="sps")
                nc.tensor.transpose(
                    tps[:], eo_sb[:, bt * 128:(bt + 1) * 128], id16_sb[:]
                )
                eo_bt = smpool.tile([128, E], F32, tag="eobt")
                nc.vector.tensor_copy(eo_bt[:], tps[:])
                prod = smpool.tile([128, E], F32, tag="prod")
                nc.vector.tensor_mul(prod[:], eo_bt[:], w_all[:, bt, :])
                y_t = smpool.tile([128, 1], F32, tag="yt")
                nc.vector.reduce_sum(y_t[:], prod[:], axis=AX.X)
                nc.sync.dma_start(y_d[bt * 128:(bt + 1) * 128, :], y_t[:])
    nc.compile()
    return nc


def round_fp32r(a):
    """Round fp32 to the FP32R format: 11-bit mantissa, RNE, low 12 bits 0."""
    u = np.ascontiguousarray(a, dtype=np.float32).view(np.uint32)
    lsb = (u >> np.uint32(12)) & np.uint32(1)
    r = (u + np.uint32(0x7FF) + lsb) & np.uint32(0xFFFFF000)
    return r.view(np.float32)


def prep_inputs(x, W1, b1, W2, b2, Wg, bg):
    """Host-side data prep. Returns (shared_map, per_core_xt)."""
    f = np.float32
    # W1 [E, D, H] -> [t=(e,ht), d_in, (d_t, h_in)] so each t is one
    # contiguous 512KB block whose SBUF layout is [128 d_in, 8 d_t * 128 h]
    w1p = np.ascontiguousarray(
        W1.reshape(E, DT, 128, HT, 128).transpose(0, 3, 2, 1, 4)
        .reshape(T, 128, DT * 128).astype(f))
    w1p = round_fp32r(w1p)
    b1t = np.ascontiguousarray(
        b1.reshape(E, HT, 128).transpose(2, 0, 1).reshape(128, T).astype(f))
    w2bd = np.zeros((128, T, E), dtype=f)
    for t in range(T):
        e, ht = divmod(t, HT)
        w2bd[:, t, e] = W2[e, ht * 128:(ht + 1) * 128]
    w2bd = round_fp32r(w2bd.reshape(128, T * E))
    wgp = np.ascontiguousarray(
        Wg.reshape(DT, 128, E).transpose(1, 0, 2).reshape(128, DT * E).astype(f))
    wgp = round_fp32r(wgp)
    ebg = np.broadcast_to(np.exp(bg.astype(f))[None, :], (128, E)).copy()
    b2p = np.ascontiguousarray(b2.astype(f).reshape(E, 1))
    id16 = np.eye(E, dtype=f)
    shared = {"w1p": w1p, "b1t": b1t, "w2bd": w2bd, "wgp": wgp,
              "ebg": ebg, "b2p": b2p, "id16": id16}
    xT = round_fp32r(np.ascontiguousarray(x.astype(f).T))  # [D, B]
    xts = [np.ascontiguousarray(xT[:, c * BS:(c + 1) * BS]) for c in range(N_CORES)]
    return shared, xts


def run(inputs, trace=False):
    nc = build_bass()
    shared, xts = prep_inputs(**inputs)
    in_maps = [dict(shared, xt=xts[c]) for c in range(N_CORES)]
    res = run_bass_kernel_spmd(
        nc, in_maps, core_ids=list(range(N_CORES)), trace=trace
    )
    y = np.concatenate([r["y"] for r in res.results], axis=0)
    return y, res


def kernel(**inputs):
    y, _ = run(inputs, trace=False)
    return y


if __name__ == "__main__":
    rng = np.random.default_rng(0)
    ins = {
        "x": rng.standard_normal((B, D), dtype=np.float32),
        "W1": rng.standard_normal((E, D, H), dtype=np.float32) / 32,
        "b1": rng.standard_normal((E, H), dtype=np.float32) / 32,
        "W2": rng.standard_normal((E, H), dtype=np.float32) / 32,
        "b2": rng.standard_normal((E,), dtype=np.float32) / 32,
        "Wg": rng.standard_normal((D, E), dtype=np.float32) / 32,
        "bg": rng.standard_normal((E,), dtype=np.float32) / 32,
    }
    y = kernel(**ins)
    print("ok", y.shape, y.dtype)



# revision 3
# speedup vs baseline: 1.2036x; 1.2036x over previous
"""MoE kernel for TRN2, 8 NeuronCores, data-parallel over the batch dim.

Reference computation (B=8192, D=1024, H=1024, E=16):
    weights = softmax(x @ Wg + bg, axis=1)            # [B, E]
    h       = relu(einsum('bd,edh->beh', x, W1) + b1) # [B, E, H]
    eo      = einsum('beh,eh->be', h, W2) + b2        # [B, E]
    out     = sum(eo * weights, axis=1, keepdims=True)# [B, 1]

Strategy (v2):
  - Shard B over 8 cores (1024 rows/core); weights replicated.
  - Stage-1 GEMM in bf16 (same 1 row/cycle PE rate as fp32r, half the
    DMA/LDWEIGHTS cost): per (e, h_tile)=t of 128, psum[h=128, b=512x2]
    accumulated over 8 d-tiles from resident xT bf16.
  - ReLU+b1 via ScalarE activation into fp32 hr tiles.
  - Stage 2 (h @ W2 per expert) moved OFF the PE onto the Vector engine:
    fused acc_e = hr * w2col + acc_e (scalar_tensor_tensor), one op per t.
    The cross-partition sum of acc_e is done by 2 small PE matmuls per
    expert with a ones|e-basis stationary (f32r), accumulating all 16
    experts into one [16, 1024] psum tile (eo^T stacked on partitions).
  - Gating matmuls (bf16) interleaved into the main loop so the startup
    critical path is just the xT + first-W1 DMAs.
  - Combine: eoT + b2 -> PE-transpose 16x128 chunks -> [128b, 16e];
    multiply by gate weights, reduce along free dim -> [128, 1] -> DMA out.
"""

import numpy as np
import ml_dtypes

import concourse.bacc as bacc
import concourse.bass as bass
import concourse.mybir as mybir
from concourse import tile
from concourse.bass_utils import run_bass_kernel_spmd

B, D, H, E = 8192, 1024, 1024, 16
N_CORES = 8
BS = B // N_CORES  # 1024 batch rows per core
NB = BS // 128     # 8 b-tiles of 128
BH = 512           # half-batch moving-operand width (one psum bank)
DT = D // 128      # 8 d-tiles
HT = H // 128      # 8 h-tiles
T = E * HT         # 128 (e, h_tile) pairs

F32 = mybir.dt.float32
F32R = mybir.dt.float32r
BF16 = mybir.dt.bfloat16
AF = mybir.ActivationFunctionType
AX = mybir.AxisListType
ALU = mybir.AluOpType
NP_BF16 = ml_dtypes.bfloat16


def build_bass():
    nc = bacc.Bacc("TRN2", target_bir_lowering=False, debug=False)
    xt_d = nc.dram_tensor("xt", [D, BS], BF16, kind="ExternalInput")
    w1_d = nc.dram_tensor("w1p", [T, 128, DT * 128], BF16, kind="ExternalInput")
    wgp_d = nc.dram_tensor("wgp", [128, DT * E], BF16, kind="ExternalInput")
    b1t_d = nc.dram_tensor("b1t", [128, T], F32, kind="ExternalInput")
    w2t_d = nc.dram_tensor("w2t", [128, T], F32, kind="ExternalInput")
    ebg_d = nc.dram_tensor("ebg", [128, E], F32, kind="ExternalInput")
    b2_d = nc.dram_tensor("b2p", [E, 1], F32, kind="ExternalInput")
    id16_d = nc.dram_tensor("id16", [E, E], F32, kind="ExternalInput")
    sel_d = nc.dram_tensor("sel", [128, E * E], F32R, kind="ExternalInput")
    y_d = nc.dram_tensor("y", [BS, 1], F32, kind="ExternalOutput")

    with tile.TileContext(nc) as tc:
        with (
            tc.tile_pool(name="const", bufs=1) as cpool,
            tc.tile_pool(name="w1", bufs=4) as w1pool,
            tc.tile_pool(name="hrelu", bufs=4) as hpool,
            tc.tile_pool(name="sm", bufs=2) as smpool,
            tc.tile_pool(name="ps_h", bufs=2, space=bass.MemorySpace.PSUM) as psh,
            tc.tile_pool(name="ps_eo", bufs=1, space=bass.MemorySpace.PSUM) as pseo,
            tc.tile_pool(name="ps_s", bufs=2, space=bass.MemorySpace.PSUM) as pss,
        ):
            # ---- resident tensors; xt first: it gates the first matmuls ----
            xt_sb = []
            for d in range(DT):
                tl = cpool.tile([128, BS], BF16, tag=f"xt{d}")
                nc.sync.dma_start(tl[:], xt_d[d * 128:(d + 1) * 128, :])
                xt_sb.append(tl)
            wgp_sb = cpool.tile([128, DT * E], BF16, tag="wgp")
            nc.sync.dma_start(wgp_sb[:], wgp_d[:])
            b1t_sb = cpool.tile([128, T], F32, tag="b1t")
            nc.sync.dma_start(b1t_sb[:], b1t_d[:])
            w2t_sb = cpool.tile([128, T], F32, tag="w2t")
            nc.sync.dma_start(w2t_sb[:], w2t_d[:])
            ebg_sb = cpool.tile([128, E], F32, tag="ebg")
            nc.sync.dma_start(ebg_sb[:], ebg_d[:])
            b2_sb = cpool.tile([E, 1], F32, tag="b2")
            nc.sync.dma_start(b2_sb[:], b2_d[:])
            id16_sb = cpool.tile([E, E], F32, tag="id16")
            nc.sync.dma_start(id16_sb[:], id16_d[:])
            sel_sb = cpool.tile([128, E * E], F32R, tag="sel")
            nc.sync.dma_start(sel_sb[:], sel_d[:])
            w_all = cpool.tile([128, NB, E], F32, tag="wall")  # gate weights
            eo_sb = cpool.tile([E, BS], F32, tag="eo")         # expert outs ^T
            acc = [cpool.tile([128, BS], F32R, tag=f"acc{e}", name=f"acc{e}")
                   for e in range(E)]

            def emit_gating(bt):
                ps_g = pss.tile([128, E], F32, tag="sps")
                for d in range(DT):
                    nc.tensor.matmul(
                        ps_g[:],
                        xt_sb[d][:, bt * 128:(bt + 1) * 128],
                        wgp_sb[:, d * E:(d + 1) * E],
                        start=(d == 0), stop=(d == DT - 1),
                        skip_group_check=True,
                    )
                pexp = smpool.tile([128, E], F32, tag="pexp")
                nc.scalar.activation(pexp[:], ps_g[:], AF.Exp)
                nc.vector.tensor_mul(pexp[:], pexp[:], ebg_sb[:])
                ssum = smpool.tile([128, 1], F32, tag="ssum")
                nc.vector.reduce_sum(ssum[:], pexp[:], axis=AX.X)
                rsum = smpool.tile([128, 1], F32, tag="rsum")
                nc.vector.reciprocal(rsum[:], ssum[:])
                nc.vector.tensor_scalar_mul(w_all[:, bt, :], pexp[:], rsum[:])

            # eo^T accumulated over all 16 experts (cross-partition sums of
            # acc_e land stacked on partitions 0..15)
            eo_ps = pseo.tile([E, BS], F32)

            def emit_eo_reduce(e):
                for bh in range(2):
                    nc.tensor.matmul(
                        eo_ps[:, bh * BH:(bh + 1) * BH],
                        sel_sb[:, e * E:(e + 1) * E],
                        acc[e][:, bh * BH:(bh + 1) * BH],
                        start=(e == 0), stop=(e == E - 1),
                        skip_group_check=True,
                    )

            # ---- main loop over t=(e, h_tile) ----
            done_q = []
            for t in range(T):
                e, ht = divmod(t, HT)
                w1t = w1pool.tile([128, DT * 128], BF16, tag="w1t")
                nc.sync.dma_start(w1t[:], w1_d[t, :, :])
                ps1 = psh.tile([128, BS], F32, tag="ps1")
                for d in range(DT):
                    lhs = w1t[:, d * 128:(d + 1) * 128]
                    for bh in range(2):
                        nc.tensor.matmul(
                            ps1[:, bh * BH:(bh + 1) * BH],
                            lhs,
                            xt_sb[d][:, bh * BH:(bh + 1) * BH],
                            start=(d == 0), stop=(d == DT - 1),
                            skip_group_check=True,
                        )
                # gating blocks slotted between early iterations (all xt has
                # landed by then; avoids a serial gating chunk at startup)
                if 1 <= t <= NB:
                    emit_gating(t - 1)
                hr = hpool.tile([128, BS], F32, tag="hr")
                for bh in range(2):
                    nc.scalar.activation(
                        hr[:, bh * BH:(bh + 1) * BH],
                        ps1[:, bh * BH:(bh + 1) * BH],
                        AF.Relu,
                        bias=b1t_sb[:, t:t + 1],
                    )
                # stage 2 on the Vector engine: acc_e (+)= hr * w2[:, t]
                if ht == 0:
                    nc.vector.tensor_scalar_mul(
                        acc[e][:], hr[:], w2t_sb[:, t:t + 1])
                else:
                    nc.vector.scalar_tensor_tensor(
                        acc[e][:], hr[:], w2t_sb[:, t:t + 1], acc[e][:],
                        ALU.mult, ALU.add)
                if ht == HT - 1:
                    done_q.append(e)
                    if len(done_q) > 2:
                        emit_eo_reduce(done_q.pop(0))
            for e in done_q:
                emit_eo_reduce(e)

            # ---- combine: (eoT + b2) -> transpose -> * gates -> reduce ----
            nc.vector.tensor_scalar_add(eo_sb[:], eo_ps[:], b2_sb[:])
            for bt in range(NB):
                tps = pss.tile([128, E], F32, tag="sps")
                nc.tensor.transpose(
                    tps[:], eo_sb[:, bt * 128:(bt + 1) * 128], id16_sb[:]
                )
                eo_bt = smpool.tile([128, E], F32, tag="eobt")
                nc.vector.tensor_copy(eo_bt[:], tps[:])
                prod = smpool.tile([128, E], F32, tag="prod")
                nc.vector.tensor_mul(prod[:], eo_bt[:], w_all[:, bt, :])
                y_t = smpool.tile([128, 1], F32, tag="yt")
                nc.vector.reduce_sum(y_t[:], prod[:], axis=AX.X)
                nc.sync.dma_start(y_d[bt * 128:(bt + 1) * 128, :], y_t[:])
    nc.compile()
    return nc


def prep_inputs(x, W1, b1, W2, b2, Wg, bg):
    """Host-side data prep. Returns (shared_map, per_core_xt)."""
    f = np.float32
    # W1 [E, D, H] -> [t=(e,ht), d_in, (d_t, h_in)] so each t is one
    # contiguous 256KB bf16 block; SBUF layout [128 d_in, 8 d_t * 128 h]
    w1p = np.ascontiguousarray(
        W1.reshape(E, DT, 128, HT, 128).transpose(0, 3, 2, 1, 4)
        .reshape(T, 128, DT * 128).astype(NP_BF16))
    b1t = np.ascontiguousarray(
        b1.reshape(E, HT, 128).transpose(2, 0, 1).reshape(128, T).astype(f))
    w2t = np.ascontiguousarray(
        W2.reshape(E, HT, 128).transpose(2, 0, 1).reshape(128, T).astype(f))
    wgp = np.ascontiguousarray(
        Wg.reshape(DT, 128, E).transpose(1, 0, 2).reshape(128, DT * E)
        .astype(NP_BF16))
    ebg = np.broadcast_to(np.exp(bg.astype(f))[None, :], (128, E)).copy()
    b2p = np.ascontiguousarray(b2.astype(f).reshape(E, 1))
    id16 = np.eye(E, dtype=f)
    # sel[:, e*16:(e+1)*16] = ones(128) x e_basis(e): the stationary that
    # column-sums acc_e into psum partition row e
    sel = np.zeros((128, E, E), dtype=f)
    for e in range(E):
        sel[:, e, e] = 1.0
    sel = np.ascontiguousarray(sel.reshape(128, E * E))
    shared = {"w1p": w1p, "b1t": b1t, "w2t": w2t, "wgp": wgp,
              "ebg": ebg, "b2p": b2p, "id16": id16, "sel": sel}
    xT = np.ascontiguousarray(x.astype(f).T.astype(NP_BF16))  # [D, B]
    xts = [np.ascontiguousarray(xT[:, c * BS:(c + 1) * BS]) for c in range(N_CORES)]
    return shared, xts


def run(inputs, trace=False):
    nc = build_bass()
    shared, xts = prep_inputs(**inputs)
    in_maps = [dict(shared, xt=xts[c]) for c in range(N_CORES)]
    res = run_bass_kernel_spmd(
        nc, in_maps, core_ids=list(range(N_CORES)), trace=trace
    )
    y = np.concatenate([r["y"] for r in res.results], axis=0)
    return y, res


def kernel(**inputs):
    y, _ = run(inputs, trace=False)
    return y


if __name__ == "__main__":
    rng = np.random.default_rng(0)
    ins = {
        "x": rng.standard_normal((B, D), dtype=np.float32),
        "W1": rng.standard_normal((E, D, H), dtype=np.float32) / 32,
        "b1": rng.standard_normal((E, H), dtype=np.float32) / 32,
        "W2": rng.standard_normal((E, H), dtype=np.float32) / 32,
        "b2": rng.standard_normal((E,), dtype=np.float32) / 32,
        "Wg": rng.standard_normal((D, E), dtype=np.float32) / 32,
        "bg": rng.standard_normal((E,), dtype=np.float32) / 32,
    }
    y = kernel(**ins)
    print("ok", y.shape, y.dtype)


# revision 13
# speedup vs baseline: 1.2865x; 1.0688x over previous
"""MoE kernel for TRN2, 8 NeuronCores, data-parallel over the batch dim.

Reference computation (B=8192, D=1024, H=1024, E=16):
    weights = softmax(x @ Wg + bg, axis=1)            # [B, E]
    h       = relu(einsum('bd,edh->beh', x, W1) + b1) # [B, E, H]
    eo      = einsum('beh,eh->be', h, W2) + b2        # [B, E]
    out     = sum(eo * weights, axis=1, keepdims=True)# [B, 1]

Strategy (v2):
  - Shard B over 8 cores (1024 rows/core); weights replicated.
  - Stage-1 GEMM in bf16 (same 1 row/cycle PE rate as fp32r, half the
    DMA/LDWEIGHTS cost): per (e, h_tile)=t of 128, psum[h=128, b=512x2]
    accumulated over 8 d-tiles from resident xT bf16.
  - ReLU+b1 via ScalarE activation into fp32 hr tiles.
  - Stage 2 (h @ W2 per expert) moved OFF the PE onto the Vector engine:
    fused acc_e = hr * w2col + acc_e (scalar_tensor_tensor), one op per t.
    The cross-partition sum of acc_e is done by 2 small PE matmuls per
    expert with a ones|e-basis stationary (f32r), accumulating all 16
    experts into one [16, 1024] psum tile (eo^T stacked on partitions).
  - Gating matmuls (bf16) interleaved into the main loop so the startup
    critical path is just the xT + first-W1 DMAs.
  - Combine: eoT + b2 -> PE-transpose 16x128 chunks -> [128b, 16e];
    multiply by gate weights, reduce along free dim -> [128, 1] -> DMA out.
"""

import numpy as np
import ml_dtypes

import concourse.bacc as bacc
import concourse.bass as bass
import concourse.mybir as mybir
from concourse import tile
from concourse.bass_utils import run_bass_kernel_spmd

B, D, H, E = 8192, 1024, 1024, 16
N_CORES = 8
BS = B // N_CORES  # 1024 batch rows per core
NB = BS // 128     # 8 b-tiles of 128
BH = 512           # half-batch moving-operand width (one psum bank)
DT = D // 128      # 8 d-tiles
HT = H // 128      # 8 h-tiles
T = E * HT         # 128 (e, h_tile) pairs

F32 = mybir.dt.float32
F32R = mybir.dt.float32r
BF16 = mybir.dt.bfloat16
AF = mybir.ActivationFunctionType
AX = mybir.AxisListType
ALU = mybir.AluOpType
NP_BF16 = ml_dtypes.bfloat16


def build_bass():
    nc = bacc.Bacc("TRN2", target_bir_lowering=False, debug=False)
    xt_d = nc.dram_tensor("xt", [D, BS], BF16, kind="ExternalInput")
    w1_d = nc.dram_tensor("w1p", [T, 128, DT * 128], BF16, kind="ExternalInput")
    wgp_d = nc.dram_tensor("wgp", [128, DT * E], BF16, kind="ExternalInput")
    b1t_d = nc.dram_tensor("b1t", [128, T], F32, kind="ExternalInput")
    w2t_d = nc.dram_tensor("w2t", [128, T], F32, kind="ExternalInput")
    ebg_d = nc.dram_tensor("ebg", [128, E], F32, kind="ExternalInput")
    b2_d = nc.dram_tensor("b2p", [E, 1], F32, kind="ExternalInput")
    id16_d = nc.dram_tensor("id16", [E, E], F32, kind="ExternalInput")
    sel_d = nc.dram_tensor("sel", [128, E * E], F32R, kind="ExternalInput")
    y_d = nc.dram_tensor("y", [128, NB], F32, kind="ExternalOutput")

    with tile.TileContext(nc) as tc:
        with (
            tc.tile_pool(name="const", bufs=1) as cpool,
            tc.tile_pool(name="w1", bufs=4) as w1pool,
            tc.tile_pool(name="hrelu", bufs=4) as hpool,
            tc.tile_pool(name="sm", bufs=2) as smpool,
            tc.tile_pool(name="ps_h", bufs=2, space=bass.MemorySpace.PSUM) as psh,
            tc.tile_pool(name="ps_eo", bufs=1, space=bass.MemorySpace.PSUM) as pseo,
            tc.tile_pool(name="ps_s", bufs=2, space=bass.MemorySpace.PSUM) as pss,
        ):
            # ---- resident tensors; xt first: it gates the first matmuls.
            # Spread the xt DMA triggers over idle engines (each trigger
            # costs ~600ns of engine time) so transfers start immediately;
            # the sync engine meanwhile runs the SPMD barrier + const loads.
            xt_eng = [nc.sync, nc.sync, nc.sync, nc.sync,
                      nc.sync, nc.sync, nc.sync, nc.sync]
            xt_sb = []
            for d in range(DT):
                tl = cpool.tile([128, BS], BF16, tag=f"xt{d}", name=f"xt{d}")
                xt_eng[d].dma_start(tl[:], xt_d[d * 128:(d + 1) * 128, :])
                xt_sb.append(tl)
            wgp_sb = cpool.tile([128, DT * E], BF16, tag="wgp")
            nc.sync.dma_start(wgp_sb[:], wgp_d[:])
            b1t_sb = cpool.tile([128, T], F32, tag="b1t")
            nc.sync.dma_start(b1t_sb[:], b1t_d[:])
            w2t_sb = cpool.tile([128, T], F32, tag="w2t")
            nc.sync.dma_start(w2t_sb[:], w2t_d[:])
            ebg_sb = cpool.tile([128, E], F32, tag="ebg")
            nc.sync.dma_start(ebg_sb[:], ebg_d[:])
            b2_sb = cpool.tile([E, 1], F32, tag="b2")
            nc.sync.dma_start(b2_sb[:], b2_d[:])
            id16_sb = cpool.tile([E, E], F32, tag="id16")
            nc.sync.dma_start(id16_sb[:], id16_d[:])
            sel_sb = cpool.tile([128, E * E], F32R, tag="sel")
            nc.sync.dma_start(sel_sb[:], sel_d[:])
            w_all = cpool.tile([128, NB, E], F32, tag="wall")  # gate weights
            eo_sb = cpool.tile([E, BS], F32, tag="eo")         # expert outs ^T
            y_all = cpool.tile([128, NB], F32, tag="yall")     # per-bt outputs
            acc = [cpool.tile([128, BS], F32R, tag=f"acc{e}", name=f"acc{e}")
                   for e in range(E)]

            def emit_gating(bt):
                ps_g = pss.tile([128, E], F32, tag="sps")
                for d in range(DT):
                    nc.tensor.matmul(
                        ps_g[:],
                        xt_sb[d][:, bt * 128:(bt + 1) * 128],
                        wgp_sb[:, d * E:(d + 1) * E],
                        start=(d == 0), stop=(d == DT - 1),
                        skip_group_check=True,
                    )
                pexp = smpool.tile([128, E], F32, tag="pexp")
                nc.scalar.activation(pexp[:], ps_g[:], AF.Exp)
                nc.vector.tensor_mul(pexp[:], pexp[:], ebg_sb[:])
                ssum = smpool.tile([128, 1], F32, tag="ssum")
                nc.vector.reduce_sum(ssum[:], pexp[:], axis=AX.X)
                rsum = smpool.tile([128, 1], F32, tag="rsum")
                nc.vector.reciprocal(rsum[:], ssum[:])
                nc.vector.tensor_scalar_mul(w_all[:, bt, :], pexp[:], rsum[:])

            # eo^T accumulated over all 16 experts (cross-partition sums of
            # acc_e land stacked on partitions 0..15)
            eo_ps = pseo.tile([E, BS], F32)

            def emit_eo_reduce(e):
                for bh in range(2):
                    nc.tensor.matmul(
                        eo_ps[:, bh * BH:(bh + 1) * BH],
                        sel_sb[:, e * E:(e + 1) * E],
                        acc[e][:, bh * BH:(bh + 1) * BH],
                        start=(e == 0), stop=(e == E - 1),
                        skip_group_check=True,
                    )

            # ---- main loop over t=(e, h_tile) ----
            done_q = []
            for t in range(T):
                e, ht = divmod(t, HT)
                w1t = w1pool.tile([128, DT * 128], BF16, tag="w1t")
                nc.sync.dma_start(w1t[:], w1_d[t, :, :])
                ps1 = psh.tile([128, BS], F32, tag="ps1")
                for d in range(DT):
                    lhs = w1t[:, d * 128:(d + 1) * 128]
                    for bh in range(2):
                        nc.tensor.matmul(
                            ps1[:, bh * BH:(bh + 1) * BH],
                            lhs,
                            xt_sb[d][:, bh * BH:(bh + 1) * BH],
                            start=(d == 0), stop=(d == DT - 1),
                            skip_group_check=True,
                        )
                # gating blocks slotted between early iterations (all xt has
                # landed by then; avoids a serial gating chunk at startup)
                if 1 <= t <= NB:
                    emit_gating(t - 1)
                hr = hpool.tile([128, BS], F32, tag="hr")
                for bh in range(2):
                    nc.scalar.activation(
                        hr[:, bh * BH:(bh + 1) * BH],
                        ps1[:, bh * BH:(bh + 1) * BH],
                        AF.Relu,
                        bias=b1t_sb[:, t:t + 1],
                    )
                # stage 2 on the Vector engine: acc_e (+)= hr * w2[:, t]
                if ht == 0:
                    nc.vector.tensor_scalar_mul(
                        acc[e][:], hr[:], w2t_sb[:, t:t + 1])
                else:
                    nc.vector.scalar_tensor_tensor(
                        acc[e][:], hr[:], w2t_sb[:, t:t + 1], acc[e][:],
                        ALU.mult, ALU.add)
                if ht == HT - 1:
                    done_q.append(e)
                    if len(done_q) > 1:
                        emit_eo_reduce(done_q.pop(0))
            for e in done_q:
                emit_eo_reduce(e)

            # ---- combine: (eoT + b2) -> transpose -> * gates -> reduce ----
            nc.vector.tensor_scalar_add(eo_sb[:], eo_ps[:], b2_sb[:])
            for bt in range(NB):
                tps = pss.tile([128, E], F32, tag="sps")
                nc.tensor.transpose(
                    tps[:], eo_sb[:, bt * 128:(bt + 1) * 128], id16_sb[:]
                )
                prod = smpool.tile([128, E], F32, tag="prod")
                nc.vector.tensor_mul(prod[:], tps[:], w_all[:, bt, :])
                nc.vector.reduce_sum(y_all[:, bt:bt + 1], prod[:], axis=AX.X)
            nc.sync.dma_start(y_d[:], y_all[:])
    nc.compile()
    return nc


def prep_inputs(x, W1, b1, W2, b2, Wg, bg):
    """Host-side data prep. Returns (shared_map, per_core_xt)."""
    f = np.float32
    # W1 [E, D, H] -> [t=(e,ht), d_in, (d_t, h_in)] so each t is one
    # contiguous 256KB bf16 block; SBUF layout [128 d_in, 8 d_t * 128 h]
    w1p = np.ascontiguousarray(
        W1.reshape(E, DT, 128, HT, 128).transpose(0, 3, 2, 1, 4)
        .reshape(T, 128, DT * 128).astype(NP_BF16))
    b1t = np.ascontiguousarray(
        b1.reshape(E, HT, 128).transpose(2, 0, 1).reshape(128, T).astype(f))
    w2t = np.ascontiguousarray(
        W2.reshape(E, HT, 128).transpose(2, 0, 1).reshape(128, T).astype(f))
    wgp = np.ascontiguousarray(
        Wg.reshape(DT, 128, E).transpose(1, 0, 2).reshape(128, DT * E)
        .astype(NP_BF16))
    ebg = np.broadcast_to(np.exp(bg.astype(f))[None, :], (128, E)).copy()
    b2p = np.ascontiguousarray(b2.astype(f).reshape(E, 1))
    id16 = np.eye(E, dtype=f)
    # sel[:, e*16:(e+1)*16] = ones(128) x e_basis(e): the stationary that
    # column-sums acc_e into psum partition row e
    sel = np.zeros((128, E, E), dtype=f)
    for e in range(E):
        sel[:, e, e] = 1.0
    sel = np.ascontiguousarray(sel.reshape(128, E * E))
    shared = {"w1p": w1p, "b1t": b1t, "w2t": w2t, "wgp": wgp,
              "ebg": ebg, "b2p": b2p, "id16": id16, "sel": sel}
    xT = np.ascontiguousarray(x.astype(f).T.astype(NP_BF16))  # [D, B]
    xts = [np.ascontiguousarray(xT[:, c * BS:(c + 1) * BS]) for c in range(N_CORES)]
    return shared, xts


def run(inputs, trace=False):
    nc = build_bass()
    shared, xts = prep_inputs(**inputs)
    in_maps = [dict(shared, xt=xts[c]) for c in range(N_CORES)]
    res = run_bass_kernel_spmd(
        nc, in_maps, core_ids=list(range(N_CORES)), trace=trace
    )
    # y per core is [128, NB] with y[p, bt] = out[bt*128 + p]
    y = np.concatenate(
        [np.ascontiguousarray(r["y"].T).reshape(BS, 1) for r in res.results],
        axis=0)
    return y, res


def kernel(**inputs):
    y, _ = run(inputs, trace=False)
    return y


if __name__ == "__main__":
    rng = np.random.default_rng(0)
    ins = {
        "x": rng.standard_normal((B, D), dtype=np.float32),
        "W1": rng.standard_normal((E, D, H), dtype=np.float32) / 32,
        "b1": rng.standard_normal((E, H), dtype=np.float32) / 32,
        "W2": rng.standard_normal((E, H), dtype=np.float32) / 32,
        "b2": rng.standard_normal((E,), dtype=np.float32) / 32,
        "Wg": rng.standard_normal((D, E), dtype=np.float32) / 32,
        "bg": rng.standard_normal((E,), dtype=np.float32) / 32,
    }
    y = kernel(**ins)
    print("ok", y.shape, y.dtype)


# revision 17
# speedup vs baseline: 1.2895x; 1.0023x over previous
"""MoE kernel for TRN2, 8 NeuronCores, data-parallel over the batch dim.

Reference computation (B=8192, D=1024, H=1024, E=16):
    weights = softmax(x @ Wg + bg, axis=1)            # [B, E]
    h       = relu(einsum('bd,edh->beh', x, W1) + b1) # [B, E, H]
    eo      = einsum('beh,eh->be', h, W2) + b2        # [B, E]
    out     = sum(eo * weights, axis=1, keepdims=True)# [B, 1]

Strategy (v2):
  - Shard B over 8 cores (1024 rows/core); weights replicated.
  - Stage-1 GEMM in bf16 (same 1 row/cycle PE rate as fp32r, half the
    DMA/LDWEIGHTS cost): per (e, h_tile)=t of 128, psum[h=128, b=512x2]
    accumulated over 8 d-tiles from resident xT bf16.
  - ReLU+b1 via ScalarE activation into fp32 hr tiles.
  - Stage 2 (h @ W2 per expert) moved OFF the PE onto the Vector engine:
    fused acc_e = hr * w2col + acc_e (scalar_tensor_tensor), one op per t.
    The cross-partition sum of acc_e is done by 2 small PE matmuls per
    expert with a ones|e-basis stationary (f32r), accumulating all 16
    experts into one [16, 1024] psum tile (eo^T stacked on partitions).
  - Gating matmuls (bf16) interleaved into the main loop so the startup
    critical path is just the xT + first-W1 DMAs.
  - Combine: eoT + b2 -> PE-transpose 16x128 chunks -> [128b, 16e];
    multiply by gate weights, reduce along free dim -> [128, 1] -> DMA out.
"""

import numpy as np
import ml_dtypes

import concourse.bacc as bacc
import concourse.bass as bass
import concourse.mybir as mybir
from concourse import tile
from concourse.bass_utils import run_bass_kernel_spmd

B, D, H, E = 8192, 1024, 1024, 16
N_CORES = 8
BS = B // N_CORES  # 1024 batch rows per core
NB = BS // 128     # 8 b-tiles of 128
BH = 512           # half-batch moving-operand width (one psum bank)
DT = D // 128      # 8 d-tiles
HT = H // 128      # 8 h-tiles
T = E * HT         # 128 (e, h_tile) pairs

F32 = mybir.dt.float32
F32R = mybir.dt.float32r
BF16 = mybir.dt.bfloat16
AF = mybir.ActivationFunctionType
AX = mybir.AxisListType
ALU = mybir.AluOpType
NP_BF16 = ml_dtypes.bfloat16


def build_bass():
    nc = bacc.Bacc("TRN2", target_bir_lowering=False, debug=False)
    xt_d = nc.dram_tensor("xt", [D, BS], BF16, kind="ExternalInput")
    w1_d = nc.dram_tensor("w1p", [T, 128, DT * 128], BF16, kind="ExternalInput")
    wgp_d = nc.dram_tensor("wgp", [128, DT * E], BF16, kind="ExternalInput")
    b1t_d = nc.dram_tensor("b1t", [128, T], F32, kind="ExternalInput")
    w2t_d = nc.dram_tensor("w2t", [128, T], F32, kind="ExternalInput")
    ebg_d = nc.dram_tensor("ebg", [128, E], F32, kind="ExternalInput")
    b2_d = nc.dram_tensor("b2p", [E, 1], F32, kind="ExternalInput")
    id16_d = nc.dram_tensor("id16", [E, E], F32, kind="ExternalInput")
    sel_d = nc.dram_tensor("sel", [128, E * E], F32R, kind="ExternalInput")
    y_d = nc.dram_tensor("y", [128, NB], F32, kind="ExternalOutput")

    with tile.TileContext(nc) as tc:
        with (
            tc.tile_pool(name="const", bufs=1) as cpool,
            tc.tile_pool(name="w1", bufs=4) as w1pool,
            tc.tile_pool(name="hrelu", bufs=4) as hpool,
            tc.tile_pool(name="sm", bufs=2) as smpool,
            tc.tile_pool(name="ps_h", bufs=2, space=bass.MemorySpace.PSUM) as psh,
            tc.tile_pool(name="ps_eo", bufs=1, space=bass.MemorySpace.PSUM) as pseo,
            tc.tile_pool(name="ps_s", bufs=2, space=bass.MemorySpace.PSUM) as pss,
        ):
            # ---- resident tensors; xt first: it gates the first matmuls.
            # Spread the xt DMA triggers over idle engines (each trigger
            # costs ~600ns of engine time) so transfers start immediately;
            # the sync engine meanwhile runs the SPMD barrier + const loads.
            xt_eng = [nc.gpsimd, nc.gpsimd, nc.scalar, nc.scalar,
                      nc.gpsimd, nc.scalar, nc.gpsimd, nc.scalar]
            xt_sb = []
            for d in range(DT):
                tl = cpool.tile([128, BS], BF16, tag=f"xt{d}", name=f"xt{d}")
                xt_eng[d].dma_start(tl[:], xt_d[d * 128:(d + 1) * 128, :])
                xt_sb.append(tl)
            wgp_sb = cpool.tile([128, DT * E], BF16, tag="wgp")
            nc.sync.dma_start(wgp_sb[:], wgp_d[:])
            b1t_sb = cpool.tile([128, T], F32, tag="b1t")
            nc.sync.dma_start(b1t_sb[:], b1t_d[:])
            w2t_sb = cpool.tile([128, T], F32, tag="w2t")
            nc.sync.dma_start(w2t_sb[:], w2t_d[:])
            ebg_sb = cpool.tile([128, E], F32, tag="ebg")
            nc.sync.dma_start(ebg_sb[:], ebg_d[:])
            b2_sb = cpool.tile([E, 1], F32, tag="b2")
            nc.sync.dma_start(b2_sb[:], b2_d[:])
            id16_sb = cpool.tile([E, E], F32, tag="id16")
            nc.sync.dma_start(id16_sb[:], id16_d[:])
            sel_sb = cpool.tile([128, E * E], F32R, tag="sel")
            nc.sync.dma_start(sel_sb[:], sel_d[:])
            w_all = cpool.tile([128, NB, E], F32, tag="wall")  # gate weights
            eo_sb = cpool.tile([E, BS], F32, tag="eo")         # expert outs ^T
            y_all = cpool.tile([128, NB], F32, tag="yall")     # per-bt outputs
            acc = [cpool.tile([128, BS], F32R, tag=f"acc{e}", name=f"acc{e}")
                   for e in range(E)]

            def emit_gating(bt):
                ps_g = pss.tile([128, E], F32, tag="sps")
                for d in range(DT):
                    nc.tensor.matmul(
                        ps_g[:],
                        xt_sb[d][:, bt * 128:(bt + 1) * 128],
                        wgp_sb[:, d * E:(d + 1) * E],
                        start=(d == 0), stop=(d == DT - 1),
                        skip_group_check=True,
                    )
                pexp = smpool.tile([128, E], F32, tag="pexp")
                nc.scalar.activation(pexp[:], ps_g[:], AF.Exp)
                nc.vector.tensor_mul(pexp[:], pexp[:], ebg_sb[:])
                ssum = smpool.tile([128, 1], F32, tag="ssum")
                nc.vector.reduce_sum(ssum[:], pexp[:], axis=AX.X)
                rsum = smpool.tile([128, 1], F32, tag="rsum")
                nc.vector.reciprocal(rsum[:], ssum[:])
                nc.vector.tensor_scalar_mul(w_all[:, bt, :], pexp[:], rsum[:])

            # eo^T accumulated over all 16 experts (cross-partition sums of
            # acc_e land stacked on partitions 0..15)
            eo_ps = pseo.tile([E, BS], F32)

            def emit_eo_reduce(e):
                for bh in range(2):
                    nc.tensor.matmul(
                        eo_ps[:, bh * BH:(bh + 1) * BH],
                        sel_sb[:, e * E:(e + 1) * E],
                        acc[e][:, bh * BH:(bh + 1) * BH],
                        start=(e == 0), stop=(e == E - 1),
                        skip_group_check=True,
                    )

            # ---- main loop over t=(e, h_tile) ----
            done_q = []
            for t in range(T):
                e, ht = divmod(t, HT)
                w1t = w1pool.tile([128, DT * 128], BF16, tag="w1t")
                # first W1 tile triggered from the idle gpsimd queue so its
                # transfer races the xt loads queued behind consts on sync
                (nc.gpsimd if t == 0 else nc.sync).dma_start(
                    w1t[:], w1_d[t, :, :])
                ps1 = psh.tile([128, BS], F32, tag="ps1")
                for d in range(DT):
                    lhs = w1t[:, d * 128:(d + 1) * 128]
                    for bh in range(2):
                        nc.tensor.matmul(
                            ps1[:, bh * BH:(bh + 1) * BH],
                            lhs,
                            xt_sb[d][:, bh * BH:(bh + 1) * BH],
                            start=(d == 0), stop=(d == DT - 1),
                            skip_group_check=True,
                        )
                # gating blocks slotted between early iterations (all xt has
                # landed by then; avoids a serial gating chunk at startup)
                if 1 <= t <= NB:
                    emit_gating(t - 1)
                hr = hpool.tile([128, BS], F32, tag="hr")
                for bh in range(2):
                    nc.scalar.activation(
                        hr[:, bh * BH:(bh + 1) * BH],
                        ps1[:, bh * BH:(bh + 1) * BH],
                        AF.Relu,
                        bias=b1t_sb[:, t:t + 1],
                    )
                # stage 2 on the Vector engine: acc_e (+)= hr * w2[:, t]
                # (split by bh so each half chains off its own ReLU half)
                for bh in range(2):
                    sl = slice(bh * BH, (bh + 1) * BH)
                    if ht == 0:
                        nc.vector.tensor_scalar_mul(
                            acc[e][:, sl], hr[:, sl], w2t_sb[:, t:t + 1])
                    else:
                        nc.vector.scalar_tensor_tensor(
                            acc[e][:, sl], hr[:, sl], w2t_sb[:, t:t + 1],
                            acc[e][:, sl], ALU.mult, ALU.add)
                if ht == HT - 1:
                    done_q.append(e)
                    if len(done_q) > 1:
                        emit_eo_reduce(done_q.pop(0))
            for e in done_q:
                emit_eo_reduce(e)

            # ---- combine: (eoT + b2) -> transpose -> * gates -> reduce ----
            # b2-add split by half so the first transposes start earlier
            for bh in range(2):
                sl = slice(bh * BH, (bh + 1) * BH)
                nc.vector.tensor_scalar_add(eo_sb[:, sl], eo_ps[:, sl],
                                            b2_sb[:])
            for bt in range(NB):
                tps = pss.tile([128, E], F32, tag="sps")
                nc.tensor.transpose(
                    tps[:], eo_sb[:, bt * 128:(bt + 1) * 128], id16_sb[:]
                )
                prod = smpool.tile([128, E], F32, tag="prod")
                nc.vector.tensor_mul(prod[:], tps[:], w_all[:, bt, :])
                nc.vector.reduce_sum(y_all[:, bt:bt + 1], prod[:], axis=AX.X)
            nc.sync.dma_start(y_d[:], y_all[:])
    nc.compile()
    return nc


def prep_inputs(x, W1, b1, W2, b2, Wg, bg):
    """Host-side data prep. Returns (shared_map, per_core_xt)."""
    f = np.float32
    # W1 [E, D, H] -> [t=(e,ht), d_in, (d_t, h_in)] so each t is one
    # contiguous 256KB bf16 block; SBUF layout [128 d_in, 8 d_t * 128 h]
    w1p = np.ascontiguousarray(
        W1.reshape(E, DT, 128, HT, 128).transpose(0, 3, 2, 1, 4)
        .reshape(T, 128, DT * 128).astype(NP_BF16))
    b1t = np.ascontiguousarray(
        b1.reshape(E, HT, 128).transpose(2, 0, 1).reshape(128, T).astype(f))
    w2t = np.ascontiguousarray(
        W2.reshape(E, HT, 128).transpose(2, 0, 1).reshape(128, T).astype(f))
    wgp = np.ascontiguousarray(
        Wg.reshape(DT, 128, E).transpose(1, 0, 2).reshape(128, DT * E)
        .astype(NP_BF16))
    ebg = np.broadcast_to(np.exp(bg.astype(f))[None, :], (128, E)).copy()
    b2p = np.ascontiguousarray(b2.astype(f).reshape(E, 1))
    id16 = np.eye(E, dtype=f)
    # sel[:, e*16:(e+1)*16] = ones(128) x e_basis(e): the stationary that
    # column-sums acc_e into psum partition row e
    sel = np.zeros((128, E, E), dtype=f)
    for e in range(E):
        sel[:, e, e] = 1.0
    sel = np.ascontiguousarray(sel.reshape(128, E * E))
    shared = {"w1p": w1p, "b1t": b1t, "w2t": w2t, "wgp": wgp,
              "ebg": ebg, "b2p": b2p, "id16": id16, "sel": sel}
    xT = np.ascontiguousarray(x.astype(f).T.astype(NP_BF16))  # [D, B]
    xts = [np.ascontiguousarray(xT[:, c * BS:(c + 1) * BS]) for c in range(N_CORES)]
    return shared, xts


def run(inputs, trace=False):
    nc = build_bass()
    shared, xts = prep_inputs(**inputs)
    in_maps = [dict(shared, xt=xts[c]) for c in range(N_CORES)]
    res = run_bass_kernel_spmd(
        nc, in_maps, core_ids=list(range(N_CORES)), trace=trace
    )
    # y per core is [128, NB] with y[p, bt] = out[bt*128 + p]
    y = np.concatenate(
        [np.ascontiguousarray(r["y"].T).reshape(BS, 1) for r in res.results],
        axis=0)
    return y, res


def kernel(**inputs):
    y, _ = run(inputs, trace=False)
    return y


if __name__ == "__main__":
    rng = np.random.default_rng(0)
    ins = {
        "x": rng.standard_normal((B, D), dtype=np.float32),
        "W1": rng.standard_normal((E, D, H), dtype=np.float32) / 32,
        "b1": rng.standard_normal((E, H), dtype=np.float32) / 32,
        "W2": rng.standard_normal((E, H), dtype=np.float32) / 32,
        "b2": rng.standard_normal((E,), dtype=np.float32) / 32,
        "Wg": rng.standard_normal((D, E), dtype=np.float32) / 32,
        "bg": rng.standard_normal((E,), dtype=np.float32) / 32,
    }
    y = kernel(**ins)
    print("ok", y.shape, y.dtype)
